# revision 57
# baseline (speedup 1.0000x reference)
"""Trainium2 Bass kernel for a 2-layer GCN (+2-layer MLP head) over a
100k-node / 3.2M-edge random graph, sharded across 8 NeuronCores.

Strategy (node/graph parallel, per the sharding hint):
  - Nodes are range-partitioned across the 8 cores (12500 each). Each core
    owns the output rows (and the scatter targets) for its node range.
  - Message tables are premultiplied per layer and normalized on the src
    side (g = D^-1/2 * (h @ W)), so per-edge work is a pure row gather +
    segment-sum; the dst-side D^-1/2 and bias/relu are applied per node
    after aggregation.
  - Each core computes the table rows for its own nodes; the full table is
    replicated to every core with an AllGather (node tables are ~32x
    smaller than edge message traffic, so this beats the all-to-all of
    halo edge messages).
  - Per-edge gathers use the dma_gather ucode instruction (int16 indices,
    so the 100k-row table is addressed in 4 megablocks of 25k rows; host
    buckets each core's edges by (dst-group of 128, src-megablock)).
  - The segment-sum runs on the tensor engine: for each 128-edge chunk a
    one-hot selector S[e, slot] = (slot == dst_local) is built on the
    vector engine and matmul-accumulated into PSUM:
        aggT[f, slot] += msg[e, f]^T @ S[e, slot]
  - Tables / matmuls in fp16 (exact 0/1 selectors, fp32 PSUM accumulate),
    dense phases in fp32.

The host side only reorders / buckets / pads edge indices (graph layout,
i.e. CSR construction); all floating-point math including degrees
(indptr differences), rsqrt normalization, dense matmuls, aggregation,
biases and activations runs on the device.
"""

import sys

if "/opt/trn_rl_repo" not in sys.path:
    sys.path.insert(0, "/opt/trn_rl_repo")

import numpy as np

N_NODES = 100000
N_EDGES = 3200000
F_IN, H1, H2, H3 = 24, 128, 64, 64
N_CORES = 8
N_MEGA = 4  # src megablocks (int16 gather index range)
GB = 128  # dst group size (nodes per PSUM accumulation group)
S_BATCH = 32  # chunks per S-build vector op
GATHER_SINGLE_PACKET = True
MAX_GATHER_CHUNKS = 7  # cap: ndesc = idxs/16+1 must stay <= 64 (one packet)
N_SWDGE_QUEUES = 4  # parallel Q7 descriptor-generation queues
DMA_SCRATCH_SIZE = 81920  # deeper SWDGE rings: fewer ring-full Q7 stalls


def _cdiv(a, b):
    return -(-a // b)


# --------------------------------------------------------------------------
# Host-side graph layout (sharding + CSR bucketing + padding)
# --------------------------------------------------------------------------


def _preprocess(edge_index, n_nodes, n_cores):
    """Bucket edges per core by (dst group, src megablock); pad each bucket
    to a multiple of 128 edges and to a chunk count shared by all cores
    (the SPMD program is identical on every core).

    Returns (sched, per_core) where sched is the shared chunk schedule and
    per_core holds each core's device input arrays.
    """
    per = n_nodes // n_cores
    ngroups = _cdiv(per, GB)
    # tables are AllGathered in two halves (rank-shard rows [0,HL) and
    # [HL,per)) so the first AG overlaps the producing phase; each
    # half-table splits into 2 megablocks at the 4-rank boundary.
    hl = min(per, _cdiv(ngroups, 2) * GB)
    hc = n_cores // 2

    src = np.concatenate(
        [np.asarray(edge_index[0], dtype=np.int64), np.arange(n_nodes, dtype=np.int64)]
    )
    dst = np.concatenate(
        [np.asarray(edge_index[1], dtype=np.int64), np.arange(n_nodes, dtype=np.int64)]
    )

    deg = np.bincount(dst, minlength=n_nodes).astype(np.int64)
    indptr = np.zeros(n_nodes + 1, dtype=np.int64)
    np.cumsum(deg, out=indptr[1:])

    core_of = dst // per
    nruns = ngroups * N_MEGA

    # src -> (cell m, megablock-local table row)
    s_r = src // per
    s_l = src - s_r * per
    s_t = (s_l >= hl).astype(np.int64)
    s_row = s_r * np.where(s_t == 0, hl, per - hl) + np.where(s_t == 0, s_l, s_l - hl)
    s_hi = (s_r >= hc).astype(np.int64)
    s_m = 2 * s_t + s_hi
    s_base = np.where(s_t == 0, s_hi * (hc * hl), s_hi * (hc * (per - hl)))
    s_loc = s_row - s_base
    assert int(s_loc.max()) < 32768

    cores_runs = []  # per core: dict of run arrays
    counts = np.zeros((n_cores, nruns), dtype=np.int64)
    for k in range(n_cores):
        sel = core_of == k
        s_k = s_loc[sel]
        dloc = dst[sel] - k * per
        g = dloc // GB
        run = g * N_MEGA + s_m[sel]
        order = np.argsort(run, kind="stable")
        s_k = s_k[order]
        dloc = dloc[order]
        run = run[order]
        counts[k] = np.bincount(run, minlength=nruns)
        cores_runs.append((s_k, dloc, run))

    # shared chunk schedule: per run, chunks = max over cores
    chunks = _cdiv(np.max(counts, axis=0), 128)  # [nruns]
    run_chunk_off = np.zeros(nruns + 1, dtype=np.int64)
    np.cumsum(chunks, out=run_chunk_off[1:])
    tot_chunks = int(run_chunk_off[-1])

    sched = {
        "per": per,
        "ngroups": ngroups,
        "hl": hl,
        "chunks": chunks.reshape(ngroups, N_MEGA),
        "tot_chunks": tot_chunks,
        "min_cnt": np.min(counts, axis=0).reshape(ngroups, N_MEGA),
    }

    # split windows (gather instructions): per run, windows of up to
    # MAX_GATHER_CHUNKS chunks; shared across cores
    win_run = []  # run id per window
    win_lo = []  # start chunk (global) per window
    win_hi = []  # end chunk (global) per window
    for r in range(nruns):
        for cs in range(0, int(chunks[r]), MAX_GATHER_CHUNKS):
            cw = min(MAX_GATHER_CHUNKS, int(chunks[r]) - cs)
            win_run.append(r)
            win_lo.append(run_chunk_off[r] + cs)
            win_hi.append(run_chunk_off[r] + cs + cw)
    nwin = len(win_run)
    sched["nwin"] = nwin

    per_core = []
    for k in range(n_cores):
        s_k, dloc, run = cores_runs[k]
        cnt = counts[k]
        run_in_off = np.zeros(nruns + 1, dtype=np.int64)
        np.cumsum(cnt, out=run_in_off[1:])
        # position of each edge in the padded stream
        rank_in_run = np.arange(len(s_k)) - run_in_off[run]
        pos = run_chunk_off[run] * 128 + rank_in_run

        tot_e = tot_chunks * 128
        idx16 = np.zeros(tot_e, dtype=np.int16)  # pads gather row 0 (S masks them)
        slots = np.full(tot_e, -1.0, dtype=np.float16)
        idx16[pos] = s_k.astype(np.int16)  # already megablock-local
        slots[pos] = (dloc - (run // N_MEGA) * GB).astype(np.float16)

        idx_sb = np.tile(
            np.ascontiguousarray(idx16.reshape(-1, 16).T), (8, 1)
        )  # [128, tot_e // 16], replicated for the 8 Q7 cores
        slots_sb = np.ascontiguousarray(slots.reshape(-1, 128).T)  # [128, tot]

        # indptr slices for own nodes, partition-minor: [p, g] -> node g*128+p
        npad = ngroups * GB
        st = np.zeros(npad, dtype=np.int32)
        en = np.ones(npad, dtype=np.int32)
        nodes = np.arange(k * per, (k + 1) * per)
        st[:per] = indptr[nodes]
        en[:per] = indptr[nodes + 1]
        st_pm = np.ascontiguousarray(st.reshape(ngroups, GB).T)  # [128, ngroups]
        en_pm = np.ascontiguousarray(en.reshape(ngroups, GB).T)

        per_core.append(
            {
                "idx": idx_sb,
                "slots": slots_sb,
                "starts": st_pm,
                "ends": en_pm,
            }
        )

    return sched, per_core


# --------------------------------------------------------------------------
# Device program
# --------------------------------------------------------------------------


def _build_program(sched, n_nodes, n_cores):
    import concourse.bacc as bacc
    import concourse.mybir as mybir
    import concourse.tile as tile

    per = sched["per"]
    ngroups = sched["ngroups"]
    hl = sched["hl"]
    hc = n_cores // 2
    gh = hl // GB  # groups in the first shard half
    chunks = sched["chunks"]  # [ngroups, N_MEGA]
    tot_chunks = sched["tot_chunks"]
    f16 = mybir.dt.float16
    f32 = mybir.dt.float32
    i16 = mybir.dt.int16
    i32 = mybir.dt.int32
    AF = mybir.ActivationFunctionType
    OP = mybir.AluOpType
    rg = [list(range(n_cores))]

    nc = bacc.Bacc(
        "TRN2",
        target_bir_lowering=False,
        debug=False,
        num_devices=n_cores,
        num_swdge_queues=N_SWDGE_QUEUES,
        dynamic_dma_scratch_size=DMA_SCRATCH_SIZE,
    )
    gq = iter(range(1 << 30))  # gather counter for queue round-robin

    # ---- I/O ----
    t_xT = nc.dram_tensor("xT", [F_IN, per], f32, kind="ExternalInput")
    t_idx = nc.dram_tensor("idx", [128, tot_chunks * 8], i16, kind="ExternalInput")
    t_slots = nc.dram_tensor("slots", [128, tot_chunks], f16, kind="ExternalInput")
    t_starts = nc.dram_tensor("starts", [128, ngroups], i32, kind="ExternalInput")
    t_ends = nc.dram_tensor("ends", [128, ngroups], i32, kind="ExternalInput")
    t_W1 = nc.dram_tensor("W1", [F_IN, H1], f32, kind="ExternalInput")
    t_W2p = nc.dram_tensor("W2p", [H1, 128], f16, kind="ExternalInput")
    t_Wl1 = nc.dram_tensor("Wl1", [H2, H3], f16, kind="ExternalInput")
    t_Wl2 = nc.dram_tensor("Wl2", [H3, 1], f16, kind="ExternalInput")
    t_b1 = nc.dram_tensor("b1", [H1, 1], f32, kind="ExternalInput")
    t_b2 = nc.dram_tensor("b2", [H2, 1], f32, kind="ExternalInput")
    t_bl1 = nc.dram_tensor("bl1", [H3, 1], f32, kind="ExternalInput")
    t_bl2 = nc.dram_tensor("bl2", [1, 1], f32, kind="ExternalInput")
    t_iota = nc.dram_tensor("iota", [128, 128], f16, kind="ExternalInput")
    t_ones = nc.dram_tensor("ones", [1, 128], f16, kind="ExternalInput")
    t_ident = nc.dram_tensor("ident", [128, 128], f32, kind="ExternalInput")
    t_out = nc.dram_tensor("out", [per, 1], f32, kind="ExternalOutput")

    def wof(g):  # nodes in group g
        return min(GB, per - g * GB)

    with tile.TileContext(nc) as tc:
        with (
            tc.tile_pool(name="cst", bufs=1) as cst,
            tc.tile_pool(name="gpool", bufs=8) as gpool,
            tc.tile_pool(name="spool", bufs=4) as spool,
            tc.tile_pool(name="idxp", bufs=4) as idxp,
            tc.tile_pool(name="xp", bufs=2) as xp,
            tc.tile_pool(name="evac", bufs=3) as evac,
            tc.tile_pool(name="pagg", bufs=4, space="PSUM") as pagg,
            tc.tile_pool(name="psm", bufs=2, space="PSUM") as psm,
            tc.tile_pool(name="dram", bufs=1, space="DRAM") as dram,
        ):
            # ---- load constants / metadata ----
            slots_t = cst.tile([128, tot_chunks], f16)
            nc.sync.dma_start(slots_t[:], t_slots[:, :])

            starts_t = cst.tile([128, ngroups], i32)
            nc.sync.dma_start(starts_t[:], t_starts[:, :])
            ends_t = cst.tile([128, ngroups], i32)
            nc.sync.dma_start(ends_t[:], t_ends[:, :])
            W1_t = cst.tile([F_IN, H1], f32)
            nc.sync.dma_start(W1_t[:], t_W1[:, :])
            W2p_t = cst.tile([H1, 128], f16)
            nc.sync.dma_start(W2p_t[:], t_W2p[:, :])
            Wl1_t = cst.tile([H2, H3], f16)
            nc.sync.dma_start(Wl1_t[:], t_Wl1[:, :])
            Wl2_t = cst.tile([H3, 1], f16)
            nc.sync.dma_start(Wl2_t[:], t_Wl2[:, :])
            b1_t = cst.tile([H1, 1], f32)
            nc.sync.dma_start(b1_t[:], t_b1[:, :])
            b2_t = cst.tile([H2, 1], f32)
            nc.sync.dma_start(b2_t[:], t_b2[:, :])
            bl1_t = cst.tile([H3, 1], f32)
            nc.sync.dma_start(bl1_t[:], t_bl1[:, :])
            bl2_t = cst.tile([1, 1], f32)
            nc.sync.dma_start(bl2_t[:], t_bl2[:, :])
            iota_t = cst.tile([128, 128], f16)
            nc.sync.dma_start(iota_t[:], t_iota[:, :])
            ones_t = cst.tile([1, 128], f16)
            nc.sync.dma_start(ones_t[:], t_ones[:, :])
            ident_t = cst.tile([128, 128], f32)
            nc.sync.dma_start(ident_t[:], t_ident[:, :])

            # ---- normalization: dis = (deg)^-1/2 for own nodes ----
            deg_i = cst.tile([128, ngroups], i32)
            nc.vector.tensor_tensor(
                out=deg_i[:], in0=ends_t[:], in1=starts_t[:], op=OP.subtract
            )
            deg_f = cst.tile([128, ngroups], f32)
            nc.vector.tensor_copy(out=deg_f[:], in_=deg_i[:])
            rdeg = cst.tile([128, ngroups], f32)
            nc.vector.reciprocal(out=rdeg[:], in_=deg_f[:])
            dis_pm = cst.tile([128, ngroups], f32)
            nc.scalar.activation(dis_pm[:], rdeg[:], AF.Sqrt)

            # dis transposed: row g holds dis[g*128 : g*128+128]
            ps_dT = psm.tile([ngroups, 128], f32, tag="sm")
            nc.tensor.transpose(out=ps_dT[:], in_=dis_pm[:], identity=ident_t[:])
            dis_pmT = cst.tile([ngroups, 128], f32)
            nc.vector.tensor_copy(out=dis_pmT[:], in_=ps_dT[:])
            # flatten to one partition-0 row (node-ordered) via reshaping DMA
            dis_pmT16 = cst.tile([ngroups, 128], f16)
            nc.vector.tensor_copy(out=dis_pmT16[:], in_=dis_pmT[:])
            dis_row = cst.tile([1, ngroups * 128], f16)
            nc.sync.dma_start(dis_row[0:1, :], dis_pmT16[:])

            # dis broadcast across partitions: dis_bcast[p, n] = dis[n]
            dis_bc = cst.tile([128, per], f16)
            for g in range(ngroups):
                w = wof(g)
                ps_b = psm.tile([128, w], f32, tag="sm")
                nc.tensor.matmul(
                    ps_b[:],
                    lhsT=ones_t[:, :128],
                    rhs=dis_row[0:1, g * GB : g * GB + w],
                )
                nc.vector.tensor_copy(
                    out=dis_bc[:, g * GB : g * GB + w], in_=ps_b[:]
                )

            # ---- internal DRAM tables ----
            g1_shard_a = dram.tile([hl, H1], f16)
            g1_shard_b = dram.tile([per - hl, H1], f16)
            g1_full_a = dram.tile([n_cores * hl, H1], f16, addr_space="Shared")
            g1_full_b = dram.tile(
                [n_cores * (per - hl), H1], f16, addr_space="Shared"
            )
            g2_shard_a = dram.tile([hl, 128], f16)
            g2_shard_b = dram.tile([per - hl, 128], f16)
            g2_full_a = dram.tile([n_cores * hl, 128], f16, addr_space="Shared")
            g2_full_b = dram.tile(
                [n_cores * (per - hl), 128], f16, addr_space="Shared"
            )

            def shard_dst(sh_a, sh_b, g):
                return (sh_a, g * GB) if g < gh else (sh_b, (g - gh) * GB)

            # ---- phase A: g1 = dis * (x @ W1), own rows, then AllGather ----
            # batched loads/stores: phase A's many small HWDGE DMAs would
            # otherwise serialize on the SP queue and delay the first AG
            WB = 4  # groups per shard-write DMA
            XB = 8  # groups per x-load DMA
            gfull = (ngroups * GB - GB + wof(ngroups - 1)) // (XB * GB) * XB
            x_tiles = {}
            for gb in range(0, gfull, XB):
                gn = min(XB, gfull - gb)
                x_b = xp.tile([F_IN, XB * GB], f32, tag="xg")
                nc.sync.dma_start(
                    x_b[:, : gn * GB], t_xT[:, gb * GB : (gb + gn) * GB]
                )
                for g in range(gb, gb + gn):
                    x_tiles[g] = (x_b, (g - gb) * GB)
            for g in range(gfull, ngroups):
                w = wof(g)
                x_g = xp.tile([F_IN, GB], f32, tag="xgt")
                nc.sync.dma_start(x_g[:, :w], t_xT[:, g * GB : g * GB + w])
                x_tiles[g] = (x_g, 0)
            # write batches: consecutive full-width groups within one half
            batches = []
            for s0, s1 in ((0, gh), (gh, ngroups)):
                g = s0
                while g < s1:
                    if g + WB <= s1 and all(
                        wof(x) == GB for x in range(g, g + WB)
                    ):
                        batches.append(list(range(g, g + WB)))
                        g += WB
                    else:
                        batches.append([g])
                        g += 1
            for bg in batches:
                if len(bg) > 1:
                    g1ev = evac.tile([GB, WB, H1], f16, tag="g1ev")
                    for c, g in enumerate(bg):
                        x_b, xo = x_tiles[g]
                        ps_x = psm.tile([GB, H1], f32, tag="sm")
                        nc.tensor.matmul(
                            ps_x[:], lhsT=x_b[:, xo : xo + GB], rhs=W1_t[:]
                        )
                        nc.scalar.activation(
                            g1ev[:, c, :],
                            ps_x[:],
                            AF.Copy,
                            scale=dis_pm[:, g : g + 1],
                        )
                    sh, r0 = shard_dst(g1_shard_a, g1_shard_b, bg[0])
                    nc.sync.dma_start(
                        sh[r0 : r0 + len(bg) * GB, :].rearrange(
                            "(c p) f -> p c f", p=GB
                        ),
                        g1ev[:, : len(bg), :],
                    )
                else:
                    g = bg[0]
                    w = wof(g)
                    x_b, xo = x_tiles[g]
                    ps_x = psm.tile([w, H1], f32, tag="sm")
                    nc.tensor.matmul(
                        ps_x[:], lhsT=x_b[:, xo : xo + w], rhs=W1_t[:]
                    )
                    g1ev1 = evac.tile([GB, H1], f16, tag="g1ev1")
                    nc.scalar.activation(
                        g1ev1[:w, :],
                        ps_x[:],
                        AF.Copy,
                        scale=dis_pm[:w, g : g + 1],
                    )
                    sh, r0 = shard_dst(g1_shard_a, g1_shard_b, g)
                    nc.sync.dma_start(sh[r0 : r0 + w, :], g1ev1[:w, :])

            nc.gpsimd.collective_compute(
                "AllGather",
                OP.bypass,
                replica_groups=rg,
                ins=[g1_shard_a[:].opt()],
                outs=[g1_full_a[:].opt()],
            )
            nc.gpsimd.collective_compute(
                "AllGather",
                OP.bypass,
                replica_groups=rg,
                ins=[g1_shard_b[:].opt()],
                outs=[g1_full_b[:].opt()],
            )



            # ---- aggregation layer (shared for L1 / L2) ----
            def aggregate(tables, felem, g):
                """Gather + segment-sum for dst group g. Returns psum tile
                [128, w] holding aggT (features on partitions)."""
                w = wof(g)
                ps = pagg.tile([128, GB], f32, tag="agg")
                crow = chunks[g]
                c0 = 0  # chunk index within group
                totc = int(crow.sum())
                # one idx load per group (covers all megablock runs)
                gbase0 = group_chunk_off[g]
                idx_g = idxp.tile([128, totc * 8], i16, tag="idx")
                nc.sync.dma_start(
                    idx_g[:], t_idx[:, gbase0 * 8 : (gbase0 + totc) * 8]
                )
                # gather tiles per megablock run
                gts = []
                for m in range(N_MEGA):
                    cm = int(crow[m])
                    if cm == 0:
                        gts.append(None)
                        continue
                    goff = gbase0 + c0
                    tbl = tables[m // 2]
                    hrows = hl if m // 2 == 0 else per - hl
                    tb0 = (m % 2) * hc * hrows
                    gt = gpool.tile([128, cm, felem], f16, tag="gt")
                    # split into balanced <=MAX_GATHER_CHUNKS-chunk gathers:
                    # ucode packs one instruction's descriptors (idxs/16+1)
                    # into single DMA packets capped at 64 descriptors
                    nw = _cdiv(cm, MAX_GATHER_CHUNKS)
                    wbase, wrem = divmod(cm, nw)
                    cs = 0
                    for wj in range(nw):
                        cw = wbase + (1 if wj < wrem else 0)
                        nc.gpsimd.dma_gather(
                            gt[:, cs : cs + cw, :],
                            tbl[tb0 : tb0 + hc * hrows, :],
                            idx_g[:, (c0 + cs) * 8 : (c0 + cs + cw) * 8],
                            cw * 128,
                            cw * 128,
                            felem,
                            single_packet=GATHER_SINGLE_PACKET,
                            queue_num=next(gq) % N_SWDGE_QUEUES,
                        )
                        cs += cw
                    gts.append((gt, goff, cm))
                    c0 += cm
                # S tiles in batches over the group's chunk range
                gbase = group_chunk_off[g]
                s_tiles = {}
                for sb0 in range(0, totc, S_BATCH):
                    bw = min(S_BATCH, totc - sb0)
                    s_t = spool.tile([128, S_BATCH * 128], f16, tag="S")
                    nc.vector.tensor_tensor(
                        out=s_t[:, : bw * 128].rearrange("p (c n) -> p c n", n=128),
                        in0=slots_t[:, gbase + sb0 : gbase + sb0 + bw].to_broadcast(
                            [128, bw, 128]
                        ),
                        in1=iota_t[:]
                        .rearrange("p (u n) -> p u n", u=1)
                        .to_broadcast([128, bw, 128]),
                        op=OP.is_equal,
                    )
                    s_tiles[sb0] = s_t
                # matmul-accumulate all chunks into ps
                ci = 0  # chunk within group
                for m in range(N_MEGA):
                    if gts[m] is None:
                        continue
                    gt, goff, cm = gts[m]
                    for c in range(cm):
                        sb0 = (ci // S_BATCH) * S_BATCH
                        s_t = s_tiles[sb0]
                        off = (ci - sb0) * 128
                        nc.tensor.matmul(
                            ps[:],
                            lhsT=gt[:, c, :],
                            rhs=s_t[:, off : off + 128],
                            start=(ci == 0),
                            stop=(ci == totc - 1),
                        )
                        ci += 1
                return ps, w

            group_chunk_off = np.zeros(ngroups + 1, dtype=np.int64)
            np.cumsum(chunks.sum(axis=1), out=group_chunk_off[1:])

            # ---- phase B: L1 aggregate; h1 = relu(dis*agg + b1); g2 = dis*(h1@W2p) ----
            for g in range(ngroups):
                ps, w = aggregate((g1_full_a, g1_full_b), H1, g)
                tmp = evac.tile([128, GB], f32, tag="tmp1")
                nc.vector.scalar_tensor_tensor(
                    out=tmp[:, :w],
                    in0=ps[:, :w],
                    scalar=0.0,
                    in1=dis_bc[:, g * GB : g * GB + w],
                    op0=OP.bypass,
                    op1=OP.mult,
                )
                h1T = evac.tile([128, GB], f16, tag="h1T")
                nc.scalar.activation(h1T[:, :w], tmp[:, :w], AF.Relu, bias=b1_t[:])
                ps_g2 = psm.tile([w, 128], f32, tag="sm")
                nc.tensor.matmul(ps_g2[:], lhsT=h1T[:, :w], rhs=W2p_t[:])
                g2ev = evac.tile([GB, 128], f16, tag="g2ev")
                nc.scalar.activation(
                    g2ev[:w, :], ps_g2[:], AF.Copy, scale=dis_pm[:w, g : g + 1]
                )
                sh2, r2 = shard_dst(g2_shard_a, g2_shard_b, g)
                nc.sync.dma_start(sh2[r2 : r2 + w, :], g2ev[:w, :])

            nc.gpsimd.collective_compute(
                "AllGather",
                OP.bypass,
                replica_groups=rg,
                ins=[g2_shard_a[:].opt()],
                outs=[g2_full_a[:].opt()],
            )
            nc.gpsimd.collective_compute(
                "AllGather",
                OP.bypass,
                replica_groups=rg,
                ins=[g2_shard_b[:].opt()],
                outs=[g2_full_b[:].opt()],
            )

            # ---- phase C: L2 aggregate; h2 = relu(dis*agg + b2); MLP head ----
            OB = 4  # groups per output-write DMA
            outb = None
            for g in range(ngroups):
                if g % OB == 0:
                    outb = evac.tile([1, OB * GB], f32, tag="outb")
                ps, w = aggregate((g2_full_a, g2_full_b), 128, g)
                tmp2 = evac.tile([H2, GB], f32, tag="tmp2")
                nc.vector.scalar_tensor_tensor(
                    out=tmp2[:, :w],
                    in0=ps[:H2, :w],
                    scalar=0.0,
                    in1=dis_bc[:H2, g * GB : g * GB + w],
                    op0=OP.bypass,
                    op1=OP.mult,
                )
                h2T = evac.tile([H2, GB], f16, tag="h2T")
                nc.scalar.activation(h2T[:, :w], tmp2[:, :w], AF.Relu, bias=b2_t[:])
                ps_h3 = psm.tile([H3, w], f32, tag="sm")
                nc.tensor.matmul(ps_h3[:], lhsT=Wl1_t[:], rhs=h2T[:, :w])
                h3T = evac.tile([H3, GB], f16, tag="h3T")
                nc.scalar.activation(h3T[:, :w], ps_h3[:], AF.Relu, bias=bl1_t[:])
                ps_o = psm.tile([1, w], f32, tag="sm")
                nc.tensor.matmul(ps_o[:], lhsT=Wl2_t[:], rhs=h3T[:, :w])
                c = g % OB
                nc.scalar.activation(
                    outb[:, c * GB : c * GB + w],
                    ps_o[:],
                    AF.Identity,
                    bias=bl2_t[:],
                )
                if g == ngroups - 1 or c == OB - 1:
                    gb0 = g - c
                    span = (g - gb0) * GB + w
                    nc.sync.dma_start(
                        t_out[gb0 * GB : gb0 * GB + span, 0:1].rearrange(
                            "n u -> u n"
                        ),
                        outb[:, :span],
                    )

    nc.compile()
    return nc


# --------------------------------------------------------------------------
# Entry point
# --------------------------------------------------------------------------


def _make_in_maps(x, sched, per_core, W1, b1, W2, b2, Wl1, bl1, Wl2, bl2):
    n_cores = len(per_core)
    per = sched["per"]
    W2p = np.zeros((H1, 128), np.float16)
    W2p[:, :H2] = W2.astype(np.float16)
    iota = np.tile(np.arange(128, dtype=np.float16)[None, :], (128, 1))
    ones = np.ones((1, 128), np.float16)
    ident = np.eye(128, dtype=np.float32)
    common = {
        "W1": np.ascontiguousarray(W1.astype(np.float32)),
        "W2p": W2p,
        "Wl1": np.ascontiguousarray(Wl1.astype(np.float16)),
        "Wl2": np.ascontiguousarray(Wl2.astype(np.float16).reshape(H3, 1)),
        "b1": np.ascontiguousarray(b1.astype(np.float32).reshape(H1, 1)),
        "b2": np.ascontiguousarray(b2.astype(np.float32).reshape(H2, 1)),
        "bl1": np.ascontiguousarray(bl1.astype(np.float32).reshape(H3, 1)),
        "bl2": np.ascontiguousarray(bl2.astype(np.float32).reshape(1, 1)),
        "iota": iota,
        "ones": ones,
        "ident": ident,
    }
    in_maps = []
    for k in range(n_cores):
        pc = per_core[k]
        xT = np.ascontiguousarray(
            np.asarray(x[k * per : (k + 1) * per], dtype=np.float32).T
        )
        in_maps.append(
            {
                "xT": xT,
                "idx": pc["idx"],
                "slots": pc["slots"],
                "starts": pc["starts"],
                "ends": pc["ends"],
                **common,
            }
        )
    return in_maps


def run_gcn(x, edge_index, W1, b1, W2, b2, Wl1, bl1, Wl2, bl2, trace=False):
    """Build + run the SPMD kernel for the given (full) inputs. Returns
    (output [N,1] float32, BassKernelResults)."""
    from concourse.bass_utils import run_bass_kernel_spmd

    import time

    n_nodes = int(np.asarray(x).shape[0])
    t0 = time.time()
    sched, per_core = _preprocess(edge_index, n_nodes, N_CORES)
    print(
        f"[gcn] preprocess {time.time() - t0:.1f}s "
        f"(tot_chunks={sched['tot_chunks']})",
        flush=True,
    )
    t0 = time.time()
    nc = _build_program(sched, n_nodes, N_CORES)
    print(f"[gcn] build+schedule {time.time() - t0:.1f}s", flush=True)
    in_maps = _make_in_maps(
        x, sched, per_core, W1, b1, W2, b2, Wl1, bl1, Wl2, bl2
    )
    t0 = time.time()
    res = run_bass_kernel_spmd(
        nc, in_maps, list(range(N_CORES)), trace=trace
    )
    print(f"[gcn] compile+run {time.time() - t0:.1f}s", flush=True)
    out = np.concatenate([res.results[k]["out"] for k in range(N_CORES)], axis=0)
    return out.astype(np.float32), res


def kernel(**inputs):
    out, _ = run_gcn(
        inputs["x"],
        inputs["edge_index"],
        inputs["W1"],
        inputs["b1"],
        inputs["W2"],
        inputs["b2"],
        inputs["Wl1"],
        inputs["bl1"],
        inputs["Wl2"],
        inputs["bl2"],
    )
    return out
